# revision 1
# baseline (speedup 1.0000x reference)
"""Trainium2 Bass kernel for NonLocalBlock (GroupNorm + 1x1 convs + HWxHW attention + residual).

Sharding: data-parallel over batch. B=8 samples -> 8 NeuronCores, one sample per core.
Per-core layout strategy:
  - x, q, k stored [C=256 -> 2 chunks of 128 partitions, N=4096 free]
  - GroupNorm fully per channel-chunk (groups of 8 channels never cross the 128 boundary);
    partition-dim aggregation/broadcast via tiny indicator matmuls on the PE
  - v computed directly transposed as vT [N -> 32 chunks of 128 partitions, C+2] with a
    ones column (softmax denominators fall out of the attn matmul) + a zero pad column
    (fp32r matmul dst must have an even free dim)
  - scores computed transposed sT[j, i] = k^T q so softmax exp is a pure elementwise op
    (no max subtraction needed: |scores/sqrt(C)| <= 16, exp fits fp32 comfortably)
  - attn^T[i, C+2] accumulated in PSUM over all j; col C = denominator; normalized by
    per-partition reciprocal; PE-transposed back to [C, i] for the output projection
  - matmul operands in float32r (1 cycle/row on PE vs 4 for fp32); producers round on
    write via bitcast output APs
"""

import os

import numpy as np

import concourse.bacc as bacc
import concourse.mybir as mybir
import concourse.tile as tile
from concourse.bass_utils import run_bass_kernel_spmd
from concourse.masks import make_identity

F32 = mybir.dt.float32
F32R = mybir.dt.float32r

B, C, H, W = 8, 256, 64, 64
HW = H * W            # 4096
P = 128
CB = C // P           # 2 channel chunks
GROUPS = 32
GPC = GROUPS // CB    # 16 groups per channel chunk
EPS = 1e-6
BAND = 512            # queries per band
NBANDS = HW // BAND   # 8
JC = HW // P          # 32 key chunks
XCH = 512             # x streaming chunk (free dim)
SCALE = float(C) ** -0.5

AF = mybir.ActivationFunctionType
ALU = mybir.AluOpType


def _build_nc(mm_dt=F32R):
    nc = bacc.Bacc(None, target_bir_lowering=False)

    xd = nc.dram_tensor("x", [C, HW], F32, kind="ExternalInput")
    wd = {
        nm: nc.dram_tensor(nm, [C, C], F32, kind="ExternalInput")
        for nm in ("wq", "wk", "wv", "wo")
    }
    vd = {
        nm: nc.dram_tensor(nm, [C], F32, kind="ExternalInput")
        for nm in ("bq", "bk", "bv", "bo", "gn_w", "gn_b")
    }
    outd = nc.dram_tensor("out", [C, HW], F32, kind="ExternalOutput")

    def mm(ap):
        # reinterpret fp32 bytes as float32r for full-rate PE matmuls
        return ap.bitcast(mm_dt) if mm_dt != ap.dtype else ap

    mmo = mm  # producers of f32r matmul operands must ROUND on write (verifier)

    with tile.TileContext(nc) as tc:
        with (
            tc.tile_pool(name="persist", bufs=1) as pp,
            tc.tile_pool(name="xpool", bufs=10) as xp,
            tc.tile_pool(name="xnpool", bufs=1) as xnp,
            tc.tile_pool(name="wload", bufs=2) as wl,
            tc.tile_pool(name="small", bufs=4) as sp,
            tc.tile_pool(name="expp", bufs=6) as ep,
            tc.tile_pool(name="attnb", bufs=2) as ab,
            tc.tile_pool(name="outp", bufs=3) as op_,
            tc.tile_pool(name="psc", bufs=4, space="PSUM") as psc,
            tc.tile_pool(name="pat", bufs=4, space="PSUM") as pat,
        ):
            # ---------------- identity + weight loads first (PE warm-up work) ----
            ident = pp.tile([P, P], F32, tag="ident", name="ident")
            make_identity(nc, ident)
            wraw = {}
            weng = {"wq": nc.gpsimd, "wk": nc.gpsimd, "wv": nc.gpsimd, "wo": nc.gpsimd}
            for nm in ("wq", "wk", "wv", "wo"):
                wsb = wl.tile([P, CB, C], F32, tag="wl", name="wl", bufs=4)
                weng[nm].dma_start(wsb, wd[nm].rearrange("(o p) c -> p o c", p=P))
                wraw[nm] = wsb

            # ---------------- x streaming loads (critical path) ----------------
            xraw = {}
            xq = [nc.sync, nc.scalar]
            for cc in range(CB):
                for nn in range(HW // XCH):
                    t = xp.tile([P, XCH], F32, tag="xl", name="xl")
                    xq[nn % len(xq)].dma_start(
                        t, xd[cc * P:(cc + 1) * P, nn * XCH:(nn + 1) * XCH])
                    xraw[(cc, nn)] = t

            ones_row = pp.tile([1, BAND], F32, tag="ones_row", name="ones_row")
            ones_stage = wl.tile([1, BAND], F32, tag="ones_stage",
                                 name="ones_stage", bufs=1)
            nc.vector.memset(ones_stage, 1.0)
            nc.vector.tensor_copy(mmo(ones_row), ones_stage)

            # per-channel vectors as [128, chunk]
            vec = {}
            for nm in ("bq", "bk", "gn_w", "gn_b"):
                t = pp.tile([P, CB], F32, tag=f"v_{nm}", name=f"v_{nm}")
                nc.gpsimd.dma_start(t, vd[nm].rearrange("(o p) -> p o", p=P))
                vec[nm] = t
            # row vectors [1, C] for bias outer products
            row = {}
            for nm in ("bv", "bo"):
                tr = wl.tile([1, C], F32, tag=f"rstage_{nm}",
                             name=f"rstage_{nm}", bufs=1)
                nc.gpsimd.dma_start(tr, vd[nm].rearrange("(a c) -> a c", a=1))
                t = pp.tile([1, C], F32, tag=f"r_{nm}", name=f"r_{nm}")
                nc.vector.tensor_copy(mmo(t), tr)
                row[nm] = t

            # group indicator G: [128, 16], G[p, g] = 1/8 iff p//8 == g (per chunk)
            Gt = pp.tile([P, GPC], F32, tag="Gt", name="Gt")
            nc.gpsimd.memset(Gt, 0.125)
            nc.gpsimd.affine_select(
                out=Gt, in_=Gt, compare_op=ALU.is_ge, fill=0.0,
                base=0, channel_multiplier=1, pattern=[[-8, GPC]],
            )
            nc.gpsimd.affine_select(
                out=Gt, in_=Gt, compare_op=ALU.is_ge, fill=0.0,
                base=7, channel_multiplier=-1, pattern=[[8, GPC]],
            )
            # broadcast indicator Bc: [16, 128], Bc[g, p] = 1 iff p//8 == g
            Bc = pp.tile([GPC, P], F32, tag="Bcast", name="Bcast")
            nc.gpsimd.memset(Bc, 1.0)
            nc.gpsimd.affine_select(
                out=Bc, in_=Bc, compare_op=ALU.is_ge, fill=0.0,
                base=0, channel_multiplier=-8, pattern=[[1, P]],
            )
            nc.gpsimd.affine_select(
                out=Bc, in_=Bc, compare_op=ALU.is_ge, fill=0.0,
                base=7, channel_multiplier=8, pattern=[[-1, P]],
            )

            # ---------------- weight loads (gpsimd queues) + PE transposes --------
            # wT[(nm, cc)] : [128 (c chunk), 256 (o)] = w[o, c].T
            wT = {}
            for nm in ("wq", "wk", "wv", "wo"):
                for cc in range(CB):
                    wT[(nm, cc)] = pp.tile([P, C], F32, tag=f"wT_{nm}{cc}",
                                           name=f"wT_{nm}{cc}")
            for nm in ("wq", "wk", "wv", "wo"):
                for oc in range(CB):
                    for cc in range(CB):
                        pt = psc.tile([P, P], F32, tag="sc", name="sc")
                        nc.tensor.transpose(
                            pt, wraw[nm][:, oc, cc * P:(cc + 1) * P], ident)
                        nc.scalar.copy(mmo(wT[(nm, cc)][:, oc * P:(oc + 1) * P]), pt)

            # ---------------- group norm, fully per channel-chunk ----------------
            xsb = []
            ab_coefs = []
            for cc in range(CB):
                st = sp.tile([P, 8, 6], F32, tag=f"st6_{cc}", name=f"st6_{cc}")
                for nn in range(HW // XCH):
                    for s8 in range(XCH // 512):
                        nc.vector.bn_stats(
                            st[:, nn * (XCH // 512) + s8, :],
                            xraw[(cc, nn)][:, s8 * 512:(s8 + 1) * 512])
                m = sp.tile([P, 2], F32, tag=f"mv{cc}", name=f"mv{cc}")
                nc.vector.bn_aggr(m, st)
                # m[:,1] := var + mean^2 = E[x^2]
                msq = sp.tile([P, 1], F32, tag="msq", name="msq")
                nc.vector.tensor_mul(msq, m[:, 0:1], m[:, 0:1])
                nc.vector.tensor_add(m[:, 1:2], m[:, 1:2], msq)

                # aggregate per-channel (mean, E[x^2]) into 16 per-group rows
                pg = pat.tile([GPC, 2], F32, tag="pat", name="pat")
                nc.tensor.matmul(pg, Gt, m, start=True, stop=True)
                sg = sp.tile([GPC, 2], F32, tag=f"sg{cc}", name=f"sg{cc}")
                nc.vector.tensor_copy(sg, pg)
                # var_g = E[x^2]_g - mean_g^2 ; rstd = 1/sqrt(var+eps)
                vg = sp.tile([GPC, 1], F32, tag=f"vg{cc}", name=f"vg{cc}")
                nc.vector.tensor_mul(vg, sg[:, 0:1], sg[:, 0:1])
                nc.vector.tensor_sub(vg, sg[:, 1:2], vg)
                epst = sp.tile([GPC, 1], F32, tag="epst", name="epst")
                nc.vector.memset(epst, EPS)
                nc.scalar.activation(vg, vg, AF.Sqrt, bias=epst)
                rstd = sp.tile([GPC, 1], F32, tag=f"rstd{cc}", name=f"rstd{cc}")
                nc.vector.reciprocal(rstd, vg)
                bcin = sp.tile([GPC, 2], F32, tag=f"bcin{cc}", name=f"bcin{cc}")
                nc.gpsimd.tensor_copy(bcin[:, 0:1], sg[:, 0:1])
                nc.gpsimd.tensor_copy(bcin[:, 1:2], rstd)

                # broadcast group (mean, rstd) back to the 128 channels
                pc = pat.tile([P, 2], F32, tag="pat", name="pat")
                nc.tensor.matmul(pc, Bc, bcin, start=True, stop=True)
                stc = sp.tile([P, 2], F32, tag=f"stc{cc}", name=f"stc{cc}")
                nc.vector.tensor_copy(stc, pc)
                # A = rstd_c * gn_w ; Bias = gn_b - mean_c * A
                A = sp.tile([P, 1], F32, tag=f"A{cc}", name=f"A{cc}")
                Bb = sp.tile([P, 1], F32, tag=f"Bb{cc}", name=f"Bb{cc}")
                nc.vector.tensor_mul(A, stc[:, 1:2], vec["gn_w"][:, cc:cc + 1])
                t1 = sp.tile([P, 1], F32, tag="t1", name="t1")
                nc.vector.tensor_mul(t1, stc[:, 0:1], A)
                nc.vector.tensor_sub(Bb, vec["gn_b"][:, cc:cc + 1], t1)
                ab_coefs.append((A, Bb))
                xn_t = xnp.tile([P, HW], F32, tag=f"xn{cc}", name=f"xn{cc}")
                xsb.append(xn_t)
            # xn = x*A + Bias, nn-major so the first q/k projection slices
            # unblock as soon as the first spatial chunk of BOTH cc is done
            for nn in range(HW // XCH):
                for cc in range(CB):
                    A, Bb = ab_coefs[cc]
                    if (nn + cc) % 2 == 0:
                        nc.scalar.activation(
                            mmo(xsb[cc][:, nn * XCH:(nn + 1) * XCH]),
                            xraw[(cc, nn)], AF.Identity, bias=Bb, scale=A)
                    else:
                        nc.vector.tensor_scalar(
                            out=mmo(xsb[cc][:, nn * XCH:(nn + 1) * XCH]),
                            in0=xraw[(cc, nn)],
                            scalar1=A, scalar2=Bb, op0=ALU.mult, op1=ALU.add,
                        )

            # ---------------- q, k projections ----------------
            qsb = [pp.tile([P, HW], F32, tag=f"q{cc}", name=f"q{cc}")
                   for cc in range(CB)]
            ksb = [pp.tile([P, HW], F32, tag=f"k{cc}", name=f"k{cc}")
                   for cc in range(CB)]
            for n8 in range(NBANDS):
                ns = slice(n8 * BAND, (n8 + 1) * BAND)
                for wname, bname, dest in (("wk", "bk", ksb), ("wq", "bq", qsb)):
                    for oc in range(CB):
                        ps = psc.tile([P, BAND], F32, tag="sc", name="sc")
                        for cc in range(CB):
                            nc.tensor.matmul(
                                ps,
                                mm(wT[(wname, cc)][:, oc * P:(oc + 1) * P]),
                                mm(xsb[cc][:, ns]),
                                start=(cc == 0), stop=(cc == 1),
                            )
                        if (n8 + oc) % 2 == 0:
                            nc.scalar.add(mmo(dest[oc][:, ns]), ps,
                                          vec[bname][:, oc:oc + 1])
                        else:
                            nc.vector.tensor_scalar_add(mmo(dest[oc][:, ns]), ps,
                                                        vec[bname][:, oc:oc + 1])

            # ---------------- vT projection (direct transposed) ----------------
            # vto[:, j, :] = [v^T[j*128:(j+1)*128, 0:256] | ones | zeros]
            vto = pp.tile([P, JC, C + 2], F32, tag="vto", name="vto")
            onespad = sp.tile([P, 2], F32, tag="onespad", name="onespad")
            nc.vector.memset(onespad[:, 0:1], 1.0)
            nc.vector.memset(onespad[:, 1:2], 0.0)
            for j in range(JC):
                nc.vector.tensor_copy(mmo(vto[:, j, C:C + 2]), onespad)
            ones1 = ones_row[:, :P]
            for j in range(JC):
                ps = psc.tile([P, C], F32, tag="sc", name="sc")
                for cc in range(CB):
                    nc.tensor.matmul(
                        ps,
                        mm(xsb[cc][:, j * P:(j + 1) * P]),
                        mm(wT[("wv", cc)]),
                        start=(cc == 0), stop=False,
                    )
                nc.tensor.matmul(ps, mm(ones1), mm(row["bv"]),
                                 start=False, stop=True)
                nc.vector.tensor_copy(mmo(vto[:, j, :C]), ps)

            # ---------------- attention bands ----------------
            for b in range(NBANDS):
                i0 = b * BAND
                pats = [pat.tile([P, C + 2], F32, tag="pat", name="pat")
                        for _ in range(4)]
                for j in range(JC):
                    ps = psc.tile([P, BAND], F32, tag="sc", name="sc")
                    for cc in range(CB):
                        nc.tensor.matmul(
                            ps,
                            mm(ksb[cc][:, j * P:(j + 1) * P]),
                            mm(qsb[cc][:, i0:i0 + BAND]),
                            start=(cc == 0), stop=(cc == 1),
                        )
                    ex = ep.tile([P, BAND], F32, tag="ex", name="ex")
                    nc.scalar.activation(mmo(ex), ps, AF.Exp, scale=SCALE)
                    for ic in range(4):
                        nc.tensor.matmul(
                            pats[ic],
                            mm(ex[:, ic * P:(ic + 1) * P]),
                            mm(vto[:, j, :]),
                            start=(j == 0), stop=(j == JC - 1),
                        )
                # drain: normalize rows, transpose back to [c, i]
                attn_band = ab.tile([P, CB, BAND], F32, tag="ab", name="ab")
                for ic in range(4):
                    rec = sp.tile([P, 1], F32, tag="rec", name="rec")
                    nc.vector.reciprocal(rec, pats[ic][:, C:C + 1])
                    atn = sp.tile([P, C], F32, tag="atn", name="atn")
                    nc.vector.tensor_scalar_mul(atn, pats[ic][:, :C], rec)
                    for cc in range(CB):
                        pt = pat.tile([P, P], F32, tag="pat", name="pat")
                        nc.tensor.transpose(pt, atn[:, cc * P:(cc + 1) * P], ident)
                        nc.vector.tensor_copy(
                            mmo(attn_band[:, cc, ic * P:(ic + 1) * P]), pt)
                # output projection + bias + residual
                for oc in range(CB):
                    po = pat.tile([P, BAND], F32, tag="pat", name="pat")
                    for cc in range(CB):
                        nc.tensor.matmul(
                            po,
                            mm(wT[("wo", cc)][:, oc * P:(oc + 1) * P]),
                            mm(attn_band[:, cc, :]),
                            start=(cc == 0), stop=False,
                        )
                    nc.tensor.matmul(po, mm(row["bo"][:, oc * P:(oc + 1) * P]),
                                     mm(ones_row), start=False, stop=True)
                    res = xp.tile([P, BAND], F32, tag="xl", name="res")
                    nc.sync.dma_start(res, xd[oc * P:(oc + 1) * P, i0:i0 + BAND])
                    ot = op_.tile([P, BAND], F32, tag="ot", name="ot")
                    nc.vector.tensor_add(ot, po, res)
                    nc.sync.dma_start(outd[oc * P:(oc + 1) * P, i0:i0 + BAND], ot)

    nc.compile()
    return nc


_NC_CACHE = {}


def get_nc(mm_dt_name=None):
    if mm_dt_name is None:
        mm_dt_name = os.environ.get("NLB_MM_DT", "f32r")
    if mm_dt_name not in _NC_CACHE:
        dt = {"f32r": F32R, "f32": F32}[mm_dt_name]
        _NC_CACHE[mm_dt_name] = _build_nc(dt)
    return _NC_CACHE[mm_dt_name]


def make_in_maps(inputs):
    x = np.ascontiguousarray(np.asarray(inputs["x"], dtype=np.float32))
    assert x.shape == (B, C, H, W), x.shape
    base = {
        nm: np.ascontiguousarray(np.asarray(inputs[nm], dtype=np.float32))
        for nm in ("wq", "bq", "wk", "bk", "wv", "bv", "wo", "bo", "gn_w", "gn_b")
    }
    return [dict(base, x=np.ascontiguousarray(x[b].reshape(C, HW))) for b in range(B)]


def kernel(**inputs) -> np.ndarray:
    nc = get_nc()
    in_maps = make_in_maps(inputs)
    res = run_bass_kernel_spmd(nc, in_maps, core_ids=list(range(B)))
    return np.stack([r["out"].reshape(C, H, W) for r in res.results])



# revision 7
# speedup vs baseline: 1.3937x; 1.3937x over previous
"""Trainium2 Bass kernel for NonLocalBlock (GroupNorm + 1x1 convs + HWxHW attention + residual).

Sharding: data-parallel over batch. B=8 samples -> 8 NeuronCores, one sample per core.
Per-core layout strategy:
  - x, q, k stored [C=256 -> 2 chunks of 128 partitions, N=4096 free]
  - GroupNorm fully per channel-chunk (groups of 8 channels never cross the 128 boundary);
    partition-dim aggregation/broadcast via tiny indicator matmuls on the PE
  - v computed directly transposed as vT [N -> 32 chunks of 128 partitions, C+2] with a
    ones column (softmax denominators fall out of the attn matmul) + a zero pad column
    (fp32r matmul dst must have an even free dim)
  - scores computed transposed sT[j, i] = k^T q so softmax exp is a pure elementwise op
    (no max subtraction needed: |scores/sqrt(C)| <= 16, exp fits fp32 comfortably)
  - attn^T[i, C+2] accumulated in PSUM over all j; col C = denominator; normalized by
    per-partition reciprocal; PE-transposed back to [C, i] for the output projection
  - matmul operands in float32r (1 cycle/row on PE vs 4 for fp32); producers round on
    write via bitcast output APs
"""

import os

import numpy as np

import concourse.bacc as bacc
import concourse.mybir as mybir
import concourse.tile as tile
from concourse.bass_utils import run_bass_kernel_spmd
from concourse.masks import make_identity

F32 = mybir.dt.float32
F32R = mybir.dt.float32r
FP8 = mybir.dt.float8e4
DR = mybir.MatmulPerfMode.DoubleRow

B, C, H, W = 8, 256, 64, 64
HW = H * W            # 4096
P = 128
CB = C // P           # 2 channel chunks
GROUPS = 32
GPC = GROUPS // CB    # 16 groups per channel chunk
EPS = 1e-6
BAND = 512            # queries per band
NBANDS = HW // BAND   # 8
JC = HW // P          # 32 key chunks
JP = JC // 2          # 16 key chunk-pairs (fp8 DoubleRow contracts 256 keys/pass)
XCH = 512             # x streaming chunk (free dim)
SCALE = float(C) ** -0.5
ESHIFT = 6.0          # constant softmax shift: keeps exp() weights in fp8 range

AF = mybir.ActivationFunctionType
ALU = mybir.AluOpType


def _build_nc(mm_dt=F32R):
    nc = bacc.Bacc(None, target_bir_lowering=False)

    xd = nc.dram_tensor("x", [C, HW], F32, kind="ExternalInput")
    wd = {
        nm: nc.dram_tensor(nm, [C, C], F32, kind="ExternalInput")
        for nm in ("wq", "wk", "wv", "wo")
    }
    vd = {
        nm: nc.dram_tensor(nm, [C], F32, kind="ExternalInput")
        for nm in ("bq", "bk", "bv", "bo", "gn_w", "gn_b")
    }
    outd = nc.dram_tensor("out", [C, HW], F32, kind="ExternalOutput")

    def mm(ap):
        # reinterpret fp32 bytes as float32r for full-rate PE matmuls
        return ap.bitcast(mm_dt) if mm_dt != ap.dtype else ap

    mmo = mm  # producers of f32r matmul operands must ROUND on write (verifier)

    with tile.TileContext(nc) as tc:
        with (
            tc.tile_pool(name="persist", bufs=1) as pp,
            tc.tile_pool(name="xpool", bufs=10) as xp,
            tc.tile_pool(name="xnpool", bufs=1) as xnp,
            tc.tile_pool(name="wload", bufs=2) as wl,
            tc.tile_pool(name="small", bufs=4) as sp,
            tc.tile_pool(name="expp", bufs=6) as ep,
            tc.tile_pool(name="attnb", bufs=2) as ab,
            tc.tile_pool(name="outp", bufs=3) as op_,
            # PSUM: "sc" slots are sized by the largest tag member ([P,2,BAND]
            # = 2 banks) x 2 bufs = 4 banks; "pat" = 4 x 1 bank. Total 8.
            tc.tile_pool(name="psc", bufs=2, space="PSUM") as psc,
            tc.tile_pool(name="pat", bufs=4, space="PSUM") as pat,
        ):
            # ---------------- identity + weight loads first (PE warm-up work) ----
            ident = pp.tile([P, P], F32, tag="ident", name="ident")
            make_identity(nc, ident)
            wraw = {}
            weng = {"wq": nc.gpsimd, "wk": nc.gpsimd, "wv": nc.gpsimd, "wo": nc.gpsimd}
            for nm in ("wq", "wk", "wv", "wo"):
                wsb = wl.tile([P, CB, C], F32, tag="wl", name="wl", bufs=4)
                weng[nm].dma_start(wsb, wd[nm].rearrange("(o p) c -> p o c", p=P))
                wraw[nm] = wsb

            # ---------------- x streaming loads (critical path) ----------------
            xraw = {}
            xq = [nc.sync, nc.scalar]
            for cc in range(CB):
                for nn in range(HW // XCH):
                    t = xp.tile([P, XCH], F32, tag="xl", name="xl")
                    xq[nn % len(xq)].dma_start(
                        t, xd[cc * P:(cc + 1) * P, nn * XCH:(nn + 1) * XCH])
                    xraw[(cc, nn)] = t

            ones_row = pp.tile([1, BAND], F32, tag="ones_row", name="ones_row")
            ones_stage = wl.tile([1, BAND], F32, tag="ones_stage",
                                 name="ones_stage", bufs=1)
            nc.vector.memset(ones_stage, 1.0)
            nc.vector.tensor_copy(mmo(ones_row), ones_stage)

            # per-channel vectors as [128, chunk]
            vec = {}
            for nm in ("bq", "bk", "gn_w", "gn_b"):
                t = pp.tile([P, CB], F32, tag=f"v_{nm}", name=f"v_{nm}")
                nc.gpsimd.dma_start(t, vd[nm].rearrange("(o p) -> p o", p=P))
                vec[nm] = t
            # row vectors [1, C] for bias outer products
            row = {}
            for nm in ("bv", "bo"):
                tr = wl.tile([1, C], F32, tag=f"rstage_{nm}",
                             name=f"rstage_{nm}", bufs=1)
                nc.gpsimd.dma_start(tr, vd[nm].rearrange("(a c) -> a c", a=1))
                t = pp.tile([1, C], F32, tag=f"r_{nm}", name=f"r_{nm}")
                nc.vector.tensor_copy(mmo(t), tr)
                row[nm] = t

            # group indicator G: [128, 16], G[p, g] = 1/8 iff p//8 == g (per chunk)
            Gt = pp.tile([P, GPC], F32, tag="Gt", name="Gt")
            nc.gpsimd.memset(Gt, 0.125)
            nc.gpsimd.affine_select(
                out=Gt, in_=Gt, compare_op=ALU.is_ge, fill=0.0,
                base=0, channel_multiplier=1, pattern=[[-8, GPC]],
            )
            nc.gpsimd.affine_select(
                out=Gt, in_=Gt, compare_op=ALU.is_ge, fill=0.0,
                base=7, channel_multiplier=-1, pattern=[[8, GPC]],
            )
            # broadcast indicator Bc: [16, 128], Bc[g, p] = 1 iff p//8 == g
            Bc = pp.tile([GPC, P], F32, tag="Bcast", name="Bcast")
            nc.gpsimd.memset(Bc, 1.0)
            nc.gpsimd.affine_select(
                out=Bc, in_=Bc, compare_op=ALU.is_ge, fill=0.0,
                base=0, channel_multiplier=-8, pattern=[[1, P]],
            )
            nc.gpsimd.affine_select(
                out=Bc, in_=Bc, compare_op=ALU.is_ge, fill=0.0,
                base=7, channel_multiplier=8, pattern=[[-1, P]],
            )

            # ---------------- weight loads (gpsimd queues) + PE transposes --------
            # wT[(nm, cc)] : [128 (c chunk), 256 (o)] = w[o, c].T
            wT = {}
            for nm in ("wq", "wk", "wv", "wo"):
                for cc in range(CB):
                    wT[(nm, cc)] = pp.tile([P, C], F32, tag=f"wT_{nm}{cc}",
                                           name=f"wT_{nm}{cc}")
            for nm in ("wq", "wk", "wv", "wo"):
                for oc in range(CB):
                    for cc in range(CB):
                        pt = psc.tile([P, P], F32, tag="sc", name="sc")
                        nc.tensor.transpose(
                            pt, wraw[nm][:, oc, cc * P:(cc + 1) * P], ident)
                        nc.scalar.copy(mmo(wT[(nm, cc)][:, oc * P:(oc + 1) * P]), pt)

            # ---------------- group norm, fully per channel-chunk ----------------
            xsb = []
            ab_coefs = []
            for cc in range(CB):
                st = sp.tile([P, 8, 6], F32, tag=f"st6_{cc}", name=f"st6_{cc}")
                for nn in range(HW // XCH):
                    for s8 in range(XCH // 512):
                        nc.vector.bn_stats(
                            st[:, nn * (XCH // 512) + s8, :],
                            xraw[(cc, nn)][:, s8 * 512:(s8 + 1) * 512])
                m = sp.tile([P, 2], F32, tag=f"mv{cc}", name=f"mv{cc}")
                nc.vector.bn_aggr(m, st)
                # m[:,1] := var + mean^2 = E[x^2]
                msq = sp.tile([P, 1], F32, tag="msq", name="msq")
                nc.vector.tensor_mul(msq, m[:, 0:1], m[:, 0:1])
                nc.vector.tensor_add(m[:, 1:2], m[:, 1:2], msq)

                # aggregate per-channel (mean, E[x^2]) into 16 per-group rows
                pg = pat.tile([GPC, 2], F32, tag="pat", name="pat")
                nc.tensor.matmul(pg, Gt, m, start=True, stop=True)
                sg = sp.tile([GPC, 2], F32, tag=f"sg{cc}", name=f"sg{cc}")
                nc.vector.tensor_copy(sg, pg)
                # var_g = E[x^2]_g - mean_g^2 ; rstd = 1/sqrt(var+eps)
                vg = sp.tile([GPC, 1], F32, tag=f"vg{cc}", name=f"vg{cc}")
                nc.vector.tensor_mul(vg, sg[:, 0:1], sg[:, 0:1])
                nc.vector.tensor_sub(vg, sg[:, 1:2], vg)
                epst = sp.tile([GPC, 1], F32, tag="epst", name="epst")
                nc.vector.memset(epst, EPS)
                nc.scalar.activation(vg, vg, AF.Sqrt, bias=epst)
                rstd = sp.tile([GPC, 1], F32, tag=f"rstd{cc}", name=f"rstd{cc}")
                nc.vector.reciprocal(rstd, vg)
                bcin = sp.tile([GPC, 2], F32, tag=f"bcin{cc}", name=f"bcin{cc}")
                nc.gpsimd.tensor_copy(bcin[:, 0:1], sg[:, 0:1])
                nc.gpsimd.tensor_copy(bcin[:, 1:2], rstd)

                # broadcast group (mean, rstd) back to the 128 channels
                pc = pat.tile([P, 2], F32, tag="pat", name="pat")
                nc.tensor.matmul(pc, Bc, bcin, start=True, stop=True)
                stc = sp.tile([P, 2], F32, tag=f"stc{cc}", name=f"stc{cc}")
                nc.vector.tensor_copy(stc, pc)
                # A = rstd_c * gn_w ; Bias = gn_b - mean_c * A
                A = sp.tile([P, 1], F32, tag=f"A{cc}", name=f"A{cc}")
                Bb = sp.tile([P, 1], F32, tag=f"Bb{cc}", name=f"Bb{cc}")
                nc.vector.tensor_mul(A, stc[:, 1:2], vec["gn_w"][:, cc:cc + 1])
                t1 = sp.tile([P, 1], F32, tag="t1", name="t1")
                nc.vector.tensor_mul(t1, stc[:, 0:1], A)
                nc.vector.tensor_sub(Bb, vec["gn_b"][:, cc:cc + 1], t1)
                ab_coefs.append((A, Bb))
                xn_t = xnp.tile([P, HW], F32, tag=f"xn{cc}", name=f"xn{cc}")
                xsb.append(xn_t)
            # xn = x*A + Bias, nn-major so the first q/k projection slices
            # unblock as soon as the first spatial chunk of BOTH cc is done
            for nn in range(HW // XCH):
                for cc in range(CB):
                    A, Bb = ab_coefs[cc]
                    if (nn + cc) % 2 == 0:
                        nc.scalar.activation(
                            mmo(xsb[cc][:, nn * XCH:(nn + 1) * XCH]),
                            xraw[(cc, nn)], AF.Identity, bias=Bb, scale=A)
                    else:
                        nc.vector.tensor_scalar(
                            out=mmo(xsb[cc][:, nn * XCH:(nn + 1) * XCH]),
                            in0=xraw[(cc, nn)],
                            scalar1=A, scalar2=Bb, op0=ALU.mult, op1=ALU.add,
                        )

            # ---------------- q, k projections (fp8 out, DoubleRow layout) ------
            # q8/k8: [o=128 partitions, 2 (o-chunk), HW] fp8 — scores contract
            # over o via fp8 DoubleRow (partition dim + the 2-dim in one pass)
            q8 = pp.tile([P, CB, HW], FP8, tag="q8", name="q8")
            k8 = pp.tile([P, CB, HW], FP8, tag="k8", name="k8")
            for n8 in range(NBANDS):
                ns = slice(n8 * BAND, (n8 + 1) * BAND)
                for wname, bname, dest in (("wk", "bk", k8), ("wq", "bq", q8)):
                    for oc in range(CB):
                        ps = psc.tile([P, BAND], F32, tag="sc", name="sc")
                        for cc in range(CB):
                            nc.tensor.matmul(
                                ps,
                                mm(wT[(wname, cc)][:, oc * P:(oc + 1) * P]),
                                mm(xsb[cc][:, ns]),
                                start=(cc == 0), stop=(cc == 1),
                            )
                        if (n8 + oc) % 2 == 0:
                            nc.scalar.add(dest[:, oc, ns], ps,
                                          vec[bname][:, oc:oc + 1])
                        else:
                            nc.vector.tensor_scalar_add(dest[:, oc, ns], ps,
                                                        vec[bname][:, oc:oc + 1])

            # ---------------- vT projection (direct transposed, fp8) ------------
            # vto8[:, p, t, :] = [v^T[(2p+t)*128:(2p+t+1)*128, 0:256] | ones | 0]
            # attn rhs slices are [128, 2, 129]: free 258 <= 512 moving limit
            vto8 = pp.tile([P, JP, 2, C + 2], FP8, tag="vto8", name="vto8")
            nc.vector.memset(vto8[:, :, :, C:C + 1], 1.0)
            nc.vector.memset(vto8[:, :, :, C + 1:C + 2], 0.0)
            eshift = pp.tile([P, 1], F32, tag="eshift", name="eshift")
            nc.gpsimd.memset(eshift, -ESHIFT)
            ones1 = ones_row[:, :P]
            for j in range(JC):
                ps = psc.tile([P, C], F32, tag="sc", name="sc")
                for cc in range(CB):
                    nc.tensor.matmul(
                        ps,
                        mm(xsb[cc][:, j * P:(j + 1) * P]),
                        mm(wT[("wv", cc)]),
                        start=(cc == 0), stop=False,
                    )
                nc.tensor.matmul(ps, mm(ones1), mm(row["bv"]),
                                 start=False, stop=True)
                nc.vector.tensor_copy(vto8[:, j // 2, j % 2, :C], ps)

            # ---------------- attention bands ----------------
            # scores computed transposed, 256 keys (one chunk-pair) at a time:
            # S[:, t, i] = k(2p+t)^T q_i ; exp -> fp8 ex8; attn accumulated via
            # fp8 DoubleRow over all 16 chunk-pairs
            for b in range(NBANDS):
                i0 = b * BAND
                pats = [pat.tile([P, C + 2], F32, tag="pat", name="pat")
                        for _ in range(4)]
                for p in range(JP):
                    ps = psc.tile([P, 2, BAND], F32, tag="sc", name="scx")
                    for t in range(2):
                        j = 2 * p + t
                        for s in range(2):
                            nc.tensor.matmul(
                                ps[:, t, s * 256:(s + 1) * 256],
                                k8[:, :, j * P:(j + 1) * P],
                                q8[:, :, i0 + s * 256:i0 + (s + 1) * 256],
                                start=True, stop=True, perf_mode=DR,
                            )
                    ex = ep.tile([P, 2, BAND], FP8, tag="ex", name="ex")
                    nc.scalar.activation(ex, ps, AF.Exp,
                                         scale=SCALE, bias=eshift)
                    for ic in range(4):
                        for h in range(2):
                            hs = slice(h * 129, (h + 1) * 129)
                            nc.tensor.matmul(
                                pats[ic][:, hs],
                                ex[:, :, ic * P:(ic + 1) * P],
                                vto8[:, p, :, hs],
                                start=(p == 0), stop=(p == JP - 1),
                                perf_mode=DR,
                            )
                # drain: normalize rows, transpose back to [c, i]
                attn_band = ab.tile([P, CB, BAND], F32, tag="ab", name="ab")
                for ic in range(4):
                    rec = sp.tile([P, 1], F32, tag="rec", name="rec")
                    nc.vector.reciprocal(rec, pats[ic][:, C:C + 1])
                    atn = sp.tile([P, C], F32, tag="atn", name="atn")
                    nc.vector.tensor_scalar_mul(atn, pats[ic][:, :C], rec)
                    for cc in range(CB):
                        pt = pat.tile([P, P], F32, tag="pat", name="pat")
                        nc.tensor.transpose(pt, atn[:, cc * P:(cc + 1) * P], ident)
                        nc.vector.tensor_copy(
                            mmo(attn_band[:, cc, ic * P:(ic + 1) * P]), pt)
                # output projection + bias + residual
                for oc in range(CB):
                    po = pat.tile([P, BAND], F32, tag="pat", name="pat")
                    for cc in range(CB):
                        nc.tensor.matmul(
                            po,
                            mm(wT[("wo", cc)][:, oc * P:(oc + 1) * P]),
                            mm(attn_band[:, cc, :]),
                            start=(cc == 0), stop=False,
                        )
                    nc.tensor.matmul(po, mm(row["bo"][:, oc * P:(oc + 1) * P]),
                                     mm(ones_row), start=False, stop=True)
                    res = xp.tile([P, BAND], F32, tag="xl", name="res")
                    nc.sync.dma_start(res, xd[oc * P:(oc + 1) * P, i0:i0 + BAND])
                    ot = op_.tile([P, BAND], F32, tag="ot", name="ot")
                    nc.vector.tensor_add(ot, po, res)
                    nc.sync.dma_start(outd[oc * P:(oc + 1) * P, i0:i0 + BAND], ot)

    nc.compile()
    return nc


_NC_CACHE = {}


def get_nc(mm_dt_name=None):
    if mm_dt_name is None:
        mm_dt_name = os.environ.get("NLB_MM_DT", "f32r")
    if mm_dt_name not in _NC_CACHE:
        dt = {"f32r": F32R, "f32": F32}[mm_dt_name]
        _NC_CACHE[mm_dt_name] = _build_nc(dt)
    return _NC_CACHE[mm_dt_name]


def make_in_maps(inputs):
    x = np.ascontiguousarray(np.asarray(inputs["x"], dtype=np.float32))
    assert x.shape == (B, C, H, W), x.shape
    base = {
        nm: np.ascontiguousarray(np.asarray(inputs[nm], dtype=np.float32))
        for nm in ("wq", "bq", "wk", "bk", "wv", "bv", "wo", "bo", "gn_w", "gn_b")
    }
    return [dict(base, x=np.ascontiguousarray(x[b].reshape(C, HW))) for b in range(B)]


def kernel(**inputs) -> np.ndarray:
    nc = get_nc()
    in_maps = make_in_maps(inputs)
    res = run_bass_kernel_spmd(nc, in_maps, core_ids=list(range(B)))
    return np.stack([r["out"].reshape(C, H, W) for r in res.results])

